# revision 30
# baseline (speedup 1.0000x reference)
"""GCNConv (SpMM + dense projection) Trainium2 Bass kernel, 8-core SPMD.

Math: out = A @ x @ W, A symmetric COO (row, col, values), N=100000 nodes,
F=128 features, 1.6M edges.

Distribution (CAGNET-style 1D row partition): core m owns destination rows
[m*12500, (m+1)*12500). x is replicated in every core's HBM; each core
gathers the source rows its edges need via dma_gather (fp16, 256B rows).

Per-core device pipeline (final: narrow scatter blocks, host-built S):
  Destinations are capacity-packed into nblk=240 blocks of <=64 slots
  under a shared tile plan (80% blocks 7 tiles / 20% blocks 6 tiles,
  light chunk rotating) so the static SPMD schedule T[b,c] carries only
  ~4.4% padding. Edges bucket per (block, chunk); chunking (4 x 25000
  rows) keeps gather indices in int16. Per group of 16 blocks (tapered
  tail groups shorten the final dependency chain):
    - 4 dma_gather calls (one per chunk, SWDGE queues 0-3; the 4 Q7
      descriptor-generator pairs run concurrently) fetch per-edge rows.
    - The scatter matrices S [128 edges x 64 dests] (value v at
      (lane, slot)) are built on the HOST and streamed from HBM - no
      per-tile vector work at all.
    - PE: per tile LDWEIGHTS(E_t, FWL) + MATMUL(S_t, 64 cols)
      accumulates z^T [128f x 64] into the block's PSUM column range.
    - Evict PSUM -> SBUF fp16 (scalar engine), project with W (PE),
      evict, DMA out. Host unpermutes slots -> rows at the end.
  The wall is SWDGE descriptor generation: ~8 ns/row per Q7 pair under
  full load, 4 pairs in parallel -> ~420 us for ~209K gathered rows;
  PE (~105 us) and everything else hide underneath.
"""
import sys

if "/opt/trn_rl_repo" not in sys.path:
    sys.path.insert(0, "/opt/trn_rl_repo")

import numpy as np
from contextlib import ExitStack

import concourse.bacc as bacc
import concourse.tile as tile
import concourse.mybir as mybir
from concourse import bass_utils

F16 = mybir.dt.float16
F32 = mybir.dt.float32
I16 = mybir.dt.int16

# ---------------------------------------------------------------- config ---
DEFAULT_CFG = dict(
    n_nodes=100000,
    feat=128,
    n_cores=8,
    npc=12500,       # destination rows per core
    n_chunk=4,       # x row chunks (gather idx must fit int16)
    ch_rows=25000,   # rows per chunk
    bdest=64,        # dests per block (= matmul rhs cols)
    nblk=240,        # blocks per core (240*64 = 15360 >= 12500)
    gblk=8,          # blocks per group (gather/evict granularity)
)


# ------------------------------------------------------- host preprocessing
def _assign_blocks(deg4, nblk, bdest, caps):
    """Capacity-planned assignment of destinations to blocks.

    caps[b, c] is the per-chunk edge budget of block b (from the shared
    tile plan). Greedy: heaviest dests first, placed into the feasible
    block with the largest worst-chunk slack; infeasible dests fall back
    to best-slack placement (rare; bumps that bucket to an extra tile).
    Returns (blk, slot) per destination.
    """
    npc = deg4.shape[0]
    tot = deg4.sum(axis=1)
    order = np.argsort(-tot, kind="stable")
    blk = np.empty(npc, np.int32)
    slot = np.empty(npc, np.int32)
    resid = caps.astype(np.float64).copy()
    nslot = np.zeros(nblk, np.int32)
    for d in order:
        need = deg4[d]
        r = resid - need
        r[nslot >= bdest] = -1e18
        score = r.min(axis=1)
        b = int(np.argmax(score))
        blk[d] = b
        slot[d] = nslot[b]
        nslot[b] += 1
        resid[b] -= need
    assert nslot.max() <= bdest, f"block overflow: {nslot.max()}"
    return blk, slot


def _preprocess(row, col, values, cfg):
    """Bucket edges per (core, block, chunk); compute the shared static tile
    schedule T[b][c]; pack per-core gather index and S streams."""
    nc_ = cfg["n_cores"]
    npc = cfg["npc"]
    chr_ = cfg["ch_rows"]
    nblk = cfg["nblk"]
    bdest = cfg["bdest"]
    gblk = cfg["gblk"]

    core = row // npc
    per_core = []
    for m in range(nc_):
        sel = np.flatnonzero(core == m)
        dl = (row[sel] - m * npc).astype(np.int64)
        cc = (col[sel] // chr_).astype(np.int64)
        lc = (col[sel] - cc * chr_).astype(np.int64)
        vv = values[sel].astype(np.float32)
        deg4 = np.bincount(dl * 4 + cc, minlength=npc * 4).reshape(npc, 4)
        # shared tile plan: 80% blocks get 7 tiles (2,2,2,1), 20% get 6
        # (2,2,1,1), rotating the light chunks; small margin keeps packing
        # feasible (overflows just bump T via the max below)
        t_plan = np.full((nblk, 4), 2, np.int64)
        r = np.arange(nblk)
        t_plan[r, r % 4] = 1
        six = r % 5 == 4
        t_plan[r[six], (r[six] + 1) % 4] = 1
        caps = t_plan * 128 - 5
        blk, slot = _assign_blocks(deg4, nblk, bdest, caps)
        counts = np.bincount(
            blk[dl].astype(np.int64) * 4 + cc, minlength=nblk * 4
        ).reshape(nblk, 4)
        per_core.append(dict(dl=dl, cc=cc, lc=lc, vv=vv, blk=blk, slot=slot,
                             counts=counts))

    # shared static schedule: tiles per (block, chunk)
    cmax = np.stack([pc["counts"] for pc in per_core]).max(axis=0)
    T = np.maximum((cmax + 127) // 128, 1).astype(np.int64)  # [nblk, 4]

    # stream layout: for group g, for chunk c, for b in group: T[b][c] tiles
    # group sizes taper at the end so the final dependency chain is short
    gsizes = [gblk] * (nblk // gblk - 1) + [gblk - 4, 4]
    assert sum(gsizes) == nblk
    gstart = np.cumsum([0] + gsizes)
    n_grp = len(gsizes)
    offs = np.zeros((nblk, 4), np.int64)     # tile offset of (b, c)
    call_tiles = np.zeros((n_grp, 4), np.int64)
    cum = 0
    for g in range(n_grp):
        for c in range(4):
            for b in range(gstart[g], gstart[g + 1]):
                offs[b, c] = cum
                cum += T[b, c]
            call_tiles[g, c] = cum - offs[gstart[g], c]
    tiles = int(cum)
    cfg["gsizes"] = gsizes

    streams = []
    for m in range(nc_):
        pc = per_core[m]
        dl, cc, lc, vv = pc["dl"], pc["cc"], pc["lc"], pc["vv"]
        blk, slot = pc["blk"], pc["slot"]
        key = blk[dl].astype(np.int64) * 4 + cc
        order = np.argsort(key, kind="stable")
        skey = key[order]
        starts = np.searchsorted(skey, np.arange(nblk * 4))
        rank = np.arange(len(skey)) - starts[skey]
        gslot = offs.reshape(-1)[skey] * 128 + rank
        assert (rank < T.reshape(-1)[skey] * 128).all()

        idx_s = np.zeros(tiles * 128, np.int16)
        idx_s[gslot] = lc[order].astype(np.int16)
        gidx = np.tile(np.ascontiguousarray(idx_s.reshape(-1, 16).T), (8, 1))

        # host-built scatter matrices: S[lane, tile, slot] = v
        s_all = np.zeros((128, tiles, bdest), np.float16)
        lane = gslot % 128
        tl = gslot // 128
        s_all[lane, tl, slot[dl][order]] = vv[order].astype(np.float16)
        s_all = np.ascontiguousarray(s_all.reshape(128, tiles * bdest))

        destmap = -np.ones(nblk * bdest, np.int64)
        destmap[blk.astype(np.int64) * bdest + slot] = np.arange(npc)
        streams.append(dict(gidx=gidx, s_all=s_all, destmap=destmap))

    return T, offs, call_tiles, tiles, streams


# ------------------------------------------------------------ device build
def _build_program(T, call_tiles, tiles, cfg):
    nc_ = cfg["n_cores"]
    nblk = cfg["nblk"]
    bdest = cfg["bdest"]
    gblk = cfg["gblk"]
    nf = cfg["feat"]
    chr_ = cfg["ch_rows"]
    gsizes = cfg["gsizes"]
    gstart = np.cumsum([0] + gsizes)
    n_grp = len(gsizes)

    nc = bacc.Bacc(
        "TRN2",
        debug=False,
        target_bir_lowering=False,
        num_devices=nc_,
        num_swdge_queues=4,
    )
    x16 = nc.dram_tensor("x16", [cfg["n_nodes"], nf], F16, kind="ExternalInput")
    w16 = nc.dram_tensor("w16", [nf, nf], F16, kind="ExternalInput")
    gidx = nc.dram_tensor("gidx", [128, tiles * 8], I16, kind="ExternalInput")
    s_hbm = nc.dram_tensor("s_all", [128, tiles * bdest], F16,
                           kind="ExternalInput")
    outT = nc.dram_tensor("outT", [128, nblk * bdest], F16,
                          kind="ExternalOutput")

    with tile.TileContext(nc) as tc, ExitStack() as ctx:
        const = ctx.enter_context(tc.tile_pool(name="const", bufs=1))
        gpools = [
            ctx.enter_context(tc.tile_pool(name=f"g{c}", bufs=3))
            for c in range(4)
        ]
        ipool = ctx.enter_context(tc.tile_pool(name="i", bufs=4))
        spool = ctx.enter_context(tc.tile_pool(name="s", bufs=3))
        pspool = ctx.enter_context(tc.tile_pool(name="ps", bufs=3, space="PSUM"))
        pzpool = ctx.enter_context(tc.tile_pool(name="pz", bufs=2, space="PSUM"))
        zbpool = ctx.enter_context(tc.tile_pool(name="zb", bufs=3))
        zopool = ctx.enter_context(tc.tile_pool(name="zo", bufs=3))

        w_t = const.tile([128, nf], F16)
        nc.sync.dma_start(w_t[:], w16[:, :])

        for g in range(n_grp):
            bs = list(range(gstart[g], gstart[g + 1]))
            ng = len(bs)
            # per-group index slice (avoids one big serial idx preload)
            gt_first = int(np.sum(call_tiles[:g]))
            gsz = int(np.sum(call_tiles[g]))
            idx_t = ipool.tile([128, gsz * 8], I16, tag="i")
            nc.scalar.dma_start(
                idx_t[:], gidx[:, gt_first * 8 : (gt_first + gsz) * 8]
            )
            # gather: one call per chunk covering the block group
            gts = []
            for c in range(4):
                sz = int(call_tiles[g, c])
                gt = gpools[c].tile([128, sz, nf], F16, tag=f"g{c}")
                t0 = int(np.sum(call_tiles[:g]) + np.sum(call_tiles[g, :c]))
                nc.gpsimd.dma_gather(
                    gt[:, :, :],
                    x16[c * chr_ : (c + 1) * chr_, :],
                    idx_t[:, (t0 - gt_first) * 8 : (t0 - gt_first + sz) * 8],
                    sz * 128,
                    sz * 128,
                    nf,
                    queue_num=c,
                    single_packet=False,
                )
                gts.append((gt, t0))

            # scatter matrices for the whole group: one DMA
            s_t = spool.tile([128, gsz, bdest], F16, tag="s")
            nc.sync.dma_start(
                s_t[:, :, :],
                s_hbm[:, gt_first * bdest : (gt_first + gsz) * bdest],
            )

            ps = pspool.tile([128, ng * bdest], F32, tag="ps")
            for bi, b in enumerate(bs):
                # last (chunk, tile) of this block
                last_ct = None
                for c in range(3, -1, -1):
                    if T[b, c] > 0:
                        last_ct = (c, int(T[b, c]) - 1)
                        break
                first = True
                for c in range(4):
                    gt, t0 = gts[c]
                    base = int(np.sum([T[bs[i], c] for i in range(bi)]))
                    for t in range(int(T[b, c])):
                        gidx_in_call = base + t
                        s_idx = t0 + gidx_in_call - gt_first
                        nc.tensor.matmul(
                            ps[:, bi * bdest : (bi + 1) * bdest],
                            gt[:, gidx_in_call, :],
                            s_t[:, s_idx, :],
                            start=first,
                            stop=(c, t) == last_ct,
                        )
                        first = False

            zb = zbpool.tile([128, ng * bdest], F16, tag="zb")
            nc.scalar.copy(zb[:], ps[:])
            zo = zopool.tile([128, ng * bdest], F16, tag="zo")
            for h in range(0, ng * bdest, 512):
                hw = min(512, ng * bdest - h)
                pz = pzpool.tile([128, hw], F32, tag="pz")
                nc.tensor.matmul(pz[:], w_t[:], zb[:, h : h + hw],
                                 start=True, stop=True)
                nc.scalar.copy(zo[:, h : h + hw], pz[:])
            nc.sync.dma_start(
                outT[:, int(gstart[g]) * bdest : int(gstart[g + 1]) * bdest],
                zo[:],
            )

    nc.compile()
    return nc


# ------------------------------------------------------------------- entry
def _run(row, col, values, x, weight, cfg, trace=False):
    row = np.asarray(row, dtype=np.int64)
    col = np.asarray(col, dtype=np.int64)
    values = np.asarray(values, dtype=np.float32)
    x = np.asarray(x, dtype=np.float32)
    weight = np.asarray(weight, dtype=np.float32)

    nc_ = cfg["n_cores"]
    npc = cfg["npc"]

    T, offs, call_tiles, tiles, streams = _preprocess(row, col, values, cfg)
    nc = _build_program(T, call_tiles, tiles, cfg)

    x16 = x.astype(np.float16)
    w16 = weight.astype(np.float16)

    in_maps = []
    for m in range(nc_):
        st = streams[m]
        in_maps.append(
            dict(x16=x16, w16=w16, gidx=st["gidx"], s_all=st["s_all"])
        )

    res = bass_utils.run_bass_kernel_spmd(
        nc, in_maps, core_ids=list(range(nc_)), trace=trace
    )

    out = np.zeros((cfg["n_nodes"], cfg["feat"]), np.float32)
    for m in range(nc_):
        oT = res.results[m]["outT"].astype(np.float32)  # [128, nblk*bdest]
        dm = streams[m]["destmap"]
        valid = dm >= 0
        out[m * npc + dm[valid]] = oT[:, valid].T
    return out, res


def kernel(row, col, values, x, weight):
    out, _ = _run(row, col, values, x, weight, DEFAULT_CFG)
    return out
